# revision 1
# baseline (speedup 1.0000x reference)
"""Trainium2 Bass kernel for per-pixel dynamic-weight 3x3 aggregation.

Computation (per sample):
    out[c, h, w] = sum_{kh,kw} xpad[c, h+kh, w+kw] * weight[c % WC, kh*3+kw, h, w]
with reflect padding (pad=1) of x.

Sharding: data-parallel over batch N=8 -> one sample per NeuronCore (8 cores).

Per-core layout (sample n):
  x:      [C=256, H=128, W=128] f32
  weight: [WC=32, KK=9, H, W]   f32
  out:    [C, H, W]             f32

Partition mapping: p = q*32 + wc, with q in 0..3 a row-quarter of the current
row-chunk and wc in 0..31 the weight channel. Free dims = (g, row, col) where
channel c = g*32 + wc. This gives every partition exactly the weight slice it
needs (no cross-partition weight replication) and keeps the 3x3 shifts in the
free dimension.

Pipeline per row-chunk of R=32 rows (4 chunks):
  - SWDGE DMA x (cast f32->f16) into xe [128, 8g, Q+2 rows, 128] (per-q DMAs)
  - ACT builds one column-shifted copy xm (xm[j] = src col j-1, width 130),
    absorbing the reflect column padding, so all 9 DVE multiplies read/write
    4-byte-aligned f16 (kw=0 reads xm[0:], kw=1 xe[0:], kw=2 xm[2:] -> 2x mode)
  - SWDGE DMA w (cast f32->f16) into [128, 9k, Q rows, 128]
  - per g-pair phase: 9 DVE tensor_tensor multiplies -> PE identity-matmul
    accumulation into PSUM (fp32) -> ACT evacuate -> HWDGE DMA store
"""

import numpy as np

import concourse.tile as tile
from concourse import bacc, mybir
from concourse.ap import AP
from concourse.bass_utils import run_bass_kernel_spmd

# Problem constants (hardcoded per contract).
N, C, H, W = 8, 256, 128, 128
WC, KK = 32, 9
G = C // WC  # 8 channel groups share one weight channel
NCORES = 8

R = 32            # rows per chunk
NCHUNK = H // R   # 4
Q = R // 4        # 8 rows handled per partition (one quarter of a chunk)
XROWS = Q + 2     # rows in the x tiles (1-row halo on each side)

FP32 = mybir.dt.float32
F16 = mybir.dt.float16

HW_ = H * W            # channel stride in x/out (elements)
WC_STRIDE = KK * HW_   # wc stride in weight

_compiled = None


def _dram_ap(t, offset, dims):
    """AP over a DRAM tensor with explicit [stride, count] dims (elements)."""
    return AP(tensor=t.ap().tensor, offset=int(offset), ap=[[int(s), int(c)] for s, c in dims])


# Note: GpSimd tensor ops serialize with DVE on real HW (shared SBUF port
# pair is an exclusive lock) — offloading multiplies there measured 310us vs
# 213us, so everything elementwise stays on DVE.


def build(reps: int = 1, do_dma: bool = True, do_compute: bool = True):
    nc = bacc.Bacc("TRN2", target_bir_lowering=False, debug=False, num_devices=1)

    x_t = nc.dram_tensor("x", [C, H, W], FP32, kind="ExternalInput")
    w_t = nc.dram_tensor("w", [WC, KK, H, W], FP32, kind="ExternalInput")
    id_t = nc.dram_tensor("ident", [128, 128], F16, kind="ExternalInput")
    o_t = nc.dram_tensor("out", [C, H, W], FP32, kind="ExternalOutput")

    with tile.TileContext(nc) as tc:
        with (
            tc.tile_pool(name="const", bufs=1) as const_pool,
            tc.tile_pool(name="xe", bufs=3) as xe_pool,
            tc.tile_pool(name="xm", bufs=2) as xm_pool,
            tc.tile_pool(name="wp", bufs=3) as w_pool,
            tc.tile_pool(name="prod", bufs=6) as prod_pool,
            tc.tile_pool(name="osb", bufs=3) as out_pool,
            tc.tile_pool(name="ps", bufs=2, space="PSUM") as psum_pool,
        ):
            ident = const_pool.tile([128, 128], F16)
            nc.sync.dma_start(ident[:], id_t.ap())

            # kw=1 taps (reading xe directly) first, so the ACT-built shifted
            # copy xm has slack to finish while DVE works on xe taps.
            K_ORDER = [1, 4, 7, 0, 3, 6, 2, 5, 8]

            def load_chunk(ch):
                r0 = ch * R
                xe = xe_pool.tile([128, G, XROWS, W], F16, tag="xe")
                wt = w_pool.tile([128, KK, Q, W], F16, tag="wt")
                xm = xm_pool.tile([128, G, XROWS, W + 2], F16, tag="xm")

                def dma_x(g0, gn):
                    # x load: [128, G, XROWS, W] f16 (col j = src col j)
                    # tile row t <- src row r0 + Q*q - 1 + t; per-q (3-dim APs)
                    for q in range(4):
                        t0 = 1 if (ch == 0 and q == 0) else 0
                        t1 = XROWS - 2 if (ch == NCHUNK - 1 and q == 3) else XROWS - 1
                        nrow = t1 - t0 + 1
                        src = _dram_ap(
                            x_t,
                            g0 * 32 * HW_ + (r0 + Q * q - 1 + t0) * W,
                            [(HW_, WC), (32 * HW_, gn), (1, nrow * W)],
                        )
                        nc.gpsimd.dma_start(
                            xe[32 * q : 32 * (q + 1), g0 : g0 + gn, t0 : t1 + 1, :], src
                        )

                def dma_x_reflect():
                    if ch == 0:  # reflect top: row -1 -> row 1
                        src = _dram_ap(x_t, 1 * W, [(HW_, WC), (32 * HW_, G), (1, W)])
                        nc.gpsimd.dma_start(xe[0:32, :, 0:1, :], src)
                    if ch == NCHUNK - 1:  # reflect bottom: 128 -> 126
                        src = _dram_ap(x_t, (H - 2) * W, [(HW_, WC), (32 * HW_, G), (1, W)])
                        nc.gpsimd.dma_start(xe[96:128, :, XROWS - 1 : XROWS, :], src)

                def dma_w(ks):
                    # w load: [128, KK, Q, W] f16, cast in DMA, per-q; ks is a
                    # (start, step, count) tap slice
                    ks0, kstep, kn = ks
                    for q in range(4):
                        src = _dram_ap(
                            w_t,
                            ks0 * HW_ + (r0 + Q * q) * W,
                            [(WC_STRIDE, WC), (kstep * HW_, kn), (1, Q * W)],
                        )
                        base = wt[32 * q : 32 * (q + 1)]
                        dst = AP(
                            tensor=base.tensor,
                            offset=base.offset + ks0 * Q * W,
                            ap=[list(base.ap[0]), [kstep * Q * W, kn], [1, Q * W]],
                        )
                        nc.gpsimd.dma_start(dst, src)

                def copy_xm(g0, gn):
                    # column-shifted copy (ACT; absorbs both reflect columns):
                    # xm[j] = src[j-1], j=0..129 (kw=0 reads xm[0:], kw=2 xm[2:],
                    # both 4B-aligned). xm[0]=src[1], xm[129]=src[126].
                    gs = slice(g0, g0 + gn)
                    nc.scalar.copy(xm[:, gs, :, 1 : W + 1], xe[:, gs, :, 0:W])
                    nc.scalar.copy(xm[:, gs, :, 0:1], xe[:, gs, :, 1:2])
                    nc.scalar.copy(
                        xm[:, gs, :, W + 1 : W + 2], xe[:, gs, :, W - 2 : W - 1]
                    )

                if do_dma:
                    dma_x(0, G)
                    dma_x_reflect()
                    dma_w((0, 1, KK))
                if do_compute:
                    copy_xm(0, G)
                return xe, xm, wt

            def run_chunk(ch, tiles):
                r0 = ch * R
                xe, xm, wt = tiles
                # per g-pair phase: multiply (DVE) + tap-sum (PE) + evac/store
                for ph in range(4):  # g in {2ph, 2ph+1}; 2048 output els/phase
                    pst = psum_pool.tile([128, 2048], FP32)
                    if do_compute:
                        for i, k in enumerate(K_ORDER):
                            kh, kw = divmod(k, 3)
                            pk = prod_pool.tile([128, 2, Q, W], F16, tag="prod")
                            wk = wt[:, k : k + 1].broadcast_to([128, 2, Q, W])
                            if kw == 1:
                                xin = xe[:, 2 * ph : 2 * ph + 2, kh : kh + Q, :]
                            else:  # kw=0 -> xm cols 0..127; kw=2 -> cols 2..129
                                xin = xm[:, 2 * ph : 2 * ph + 2, kh : kh + Q, kw : kw + W]
                            nc.vector.tensor_mul(pk[:], xin, wk)
                            rk = pk[:].rearrange("p g r c -> p (g r c)")
                            for j in range(4):
                                nc.tensor.matmul(
                                    pst[:, j * 512 : (j + 1) * 512],
                                    ident[:],
                                    rk[:, j * 512 : (j + 1) * 512],
                                    start=(i == 0),
                                    stop=(i == KK - 1),
                                )
                    osb = out_pool.tile([128, 2048], FP32)
                    if do_compute:
                        nc.scalar.copy(osb[:], pst[:])
                    for q in range(4 if do_dma else 0):
                        dst = _dram_ap(
                            o_t,
                            2 * ph * 32 * HW_ + (r0 + Q * q) * W,
                            [(HW_, WC), (32 * HW_, 2), (1, Q * W)],
                        )
                        nc.sync.dma_start(dst, osb[32 * q : 32 * (q + 1), :])

            def emit_body():
                # software-pipelined emission: prefetch chunk ch+1 before
                # the compute phases of chunk ch
                tiles = load_chunk(0)
                for ch in range(NCHUNK):
                    nxt = load_chunk(ch + 1) if ch + 1 < NCHUNK else None
                    run_chunk(ch, tiles)
                    tiles = nxt

            if reps == 1:
                emit_body()
            else:  # timing builds: repeat the whole kernel on-device
                with tc.For_i(
                    0, reps, 1,
                    hint_engines=(mybir.EngineType.PE, mybir.EngineType.DVE),
                ):
                    emit_body()

    nc.compile()
    return nc


def _get_compiled():
    global _compiled
    if _compiled is None:
        _compiled = build()
    return _compiled


def kernel(x: np.ndarray, weight: np.ndarray) -> np.ndarray:
    nc = _get_compiled()
    ident = np.eye(128, dtype=np.float16)
    in_maps = [
        {
            "x": np.ascontiguousarray(x[i], dtype=np.float32),
            "w": np.ascontiguousarray(weight[i], dtype=np.float32),
            "ident": ident,
        }
        for i in range(NCORES)
    ]
    res = run_bass_kernel_spmd(nc, in_maps, core_ids=list(range(NCORES)))
    return np.stack([res.results[i]["out"] for i in range(NCORES)], axis=0)

